# revision 26
# baseline (speedup 1.0000x reference)
import sys

sys.path.insert(0, "/opt/trn_rl_repo")

import numpy as np
import ml_dtypes

# Phi3SeerAttention, B=1 S=2048 HIDDEN=3072, H=32 q heads, HK=8 kv heads,
# D=96, gate block 64, gate hidden 128. Sharded TP over kv heads: core c
# owns kv head c and q heads 4c..4c+3; o-proj row-sharded, partials summed
# on host (the gather step).
H, HK, D, BLK, GH = 32, 8, 96, 64, 128
S, HIDDEN = 2048, 3072
G = H // HK          # 4 q heads per kv head (per core)
NB = S // BLK        # 32 gate blocks
KT = HIDDEN // 128   # 24 contraction tiles
NS = S // 512        # 4 sequence chunks of 512
NT = S // 128        # 16 t-tiles of 128
NE = HIDDEN // 512   # 6 output column chunks
NCORES = 8
THR = 0.03

_prog = None


def _build(debug=False):
    from concourse import bass, mybir, bacc
    import concourse.tile as tile
    from contextlib import ExitStack

    dt = mybir.dt
    BF, F32 = dt.bfloat16, dt.float32
    AF = mybir.ActivationFunctionType
    OP = mybir.AluOpType
    AX = mybir.AxisListType.X

    nc = bacc.Bacc()
    xt_d = nc.dram_tensor("xt", [HIDDEN, S], BF, kind="ExternalInput")
    wq_d = nc.dram_tensor("wq", [HIDDEN, G * D], BF, kind="ExternalInput")
    wk_d = nc.dram_tensor("wk", [HIDDEN, D], BF, kind="ExternalInput")
    wv_d = nc.dram_tensor("wv", [HIDDEN, D], BF, kind="ExternalInput")
    ow_d = nc.dram_tensor("ow", [G * D, HIDDEN], BF, kind="ExternalInput")
    cosq_d = nc.dram_tensor("cosq", [D, S], BF, kind="ExternalInput")
    sinq_d = nc.dram_tensor("sinq", [D, S], BF, kind="ExternalInput")
    cosk_d = nc.dram_tensor("cosk", [D, S], BF, kind="ExternalInput")
    sink_d = nc.dram_tensor("sink", [D, S], BF, kind="ExternalInput")
    rot_d = nc.dram_tensor("rot", [D, D], BF, kind="ExternalInput")
    gwq_d = nc.dram_tensor("gwq", [D, GH], F32, kind="ExternalInput")
    gwk_d = nc.dram_tensor("gwk", [2 * D, GH], F32, kind="ExternalInput")
    eye_d = nc.dram_tensor("eye32", [NB, NB], F32, kind="ExternalInput")
    emat_d = nc.dram_tensor("emat", [NB, NT * 128], F32, kind="ExternalInput")
    bcm_d = nc.dram_tensor("bcm", [NB, NB], F32, kind="ExternalInput")
    cmask_d = nc.dram_tensor("cmask", [128, 4 * 512], BF, kind="ExternalInput")
    out_d = nc.dram_tensor("out_p", [S, HIDDEN], BF, kind="ExternalOutput")

    # Raw (persistent) SBUF tensors that cross the phase-1 barrier. The two
    # TileContexts are separated by a full drain+barrier so no instruction
    # ever needs to wait on the union of all 8 DMA HW queue semaphores
    # (compute-engine instructions have a small embedded sync-wait cap).
    q_sb = nc.alloc_sbuf_tensor("q_sbuf", [D, G, S], BF)
    k_sb = nc.alloc_sbuf_tensor("k_sbuf", [D, S], BF)
    v_sb = nc.alloc_sbuf_tensor("v_sbuf", [128, NT, D + 1], BF)
    qp_sb = nc.alloc_sbuf_tensor("qp_sbuf", [D, G, NB], F32)
    km_sb = nc.alloc_sbuf_tensor("km_sbuf", [D, NB], F32)
    kx_sb = nc.alloc_sbuf_tensor("kx_sbuf", [D, NB], F32)

    # ---- context A / phase 1: QKV projection + gate pooling ----
    with tile.TileContext(nc) as tc:
        with tc.tile_pool(name="xw", bufs=1) as xw, tc.tile_pool(
            name="ps1", bufs=6, space="PSUM"
        ) as ps1:
            xt_sb = xw.tile([128, KT, S], BF)
            wq_sb = xw.tile([128, KT, G * D], BF)
            wk_sb = xw.tile([128, KT, D], BF)
            wv_sb = xw.tile([128, KT, D], BF)
            for kt in range(KT):
                r = slice(kt * 128, (kt + 1) * 128)
                nc.sync.dma_start(wq_sb[:, kt, :], wq_d[r, :])
                nc.sync.dma_start(wk_sb[:, kt, :], wk_d[r, :])
                nc.sync.dma_start(wv_sb[:, kt, :], wv_d[r, :])
            nc.vector.memset(v_sb[:, :, D : D + 1], 1.0)

            for j in range(NS):
                sl = slice(j * 512, (j + 1) * 512)
                for kt in range(KT):
                    r = slice(kt * 128, (kt + 1) * 128)
                    nc.sync.dma_start(xt_sb[:, kt, sl], xt_d[r, sl])

                for hh in range(G + 1):  # 0..3 = q heads, 4 = k
                    ps = ps1.tile([128, 512], F32)
                    pq = ps[:D, :]
                    for kt in range(KT):
                        lhsT = (
                            wq_sb[:, kt, hh * D : (hh + 1) * D]
                            if hh < G
                            else wk_sb[:, kt, :]
                        )
                        nc.tensor.matmul(
                            pq,
                            lhsT,
                            xt_sb[:, kt, sl],
                            start=(kt == 0),
                            stop=(kt == KT - 1),
                        )
                    pr = pq.rearrange("p (b w) -> p b w", w=BLK)
                    bs = slice(j * 8, (j + 1) * 8)
                    if hh < G:
                        # block SUM; 1/BLK folded into gate scale
                        nc.vector.tensor_reduce(
                            qp_sb[:, hh, bs], pr, axis=AX, op=OP.add
                        )
                        nc.scalar.copy(q_sb[:, hh, sl], pq)
                    else:
                        # block SUM; 1/BLK folded into gwk rows on host
                        nc.vector.tensor_reduce(km_sb[:, bs], pr, axis=AX, op=OP.add)
                        nc.vector.tensor_reduce(kx_sb[:, bs], pr, axis=AX, op=OP.max)
                        nc.scalar.copy(k_sb[:, sl], pq)

                for ti in range(4 * j, 4 * (j + 1)):
                    ps = ps1.tile([128, 512], F32)
                    pv = ps[:, :D]
                    for kt in range(KT):
                        nc.tensor.matmul(
                            pv,
                            xt_sb[:, kt, ti * 128 : (ti + 1) * 128],
                            wv_sb[:, kt, :],
                            start=(kt == 0),
                            stop=(kt == KT - 1),
                        )
                    nc.scalar.copy(v_sb[:, ti, :D], pv)

    # ---- context B: gate, RoPE, attention, o-projection ----
    with tile.TileContext(nc) as tc:
        with ExitStack() as ctx:
            perm = ctx.enter_context(tc.tile_pool(name="perm", bufs=1))
            mask_sb = perm.tile([128, NT, NB], BF)
            rot_sb = perm.tile([D, D], BF)
            gwq_sb = perm.tile([D, GH], F32)
            gwk_sb = perm.tile([D, 2, GH], F32)
            eye_sb = perm.tile([NB, NB], F32)
            bcm_sb = perm.tile([NB, NB], F32)
            ones_sb = perm.tile([1, 128], BF)
            attn_sb = perm.tile([D, G, S], BF)  # normalized attn output^T
            cosq_sb = perm.tile([D, S], BF)
            sinq_sb = perm.tile([D, S], BF)
            cosk_sb = perm.tile([D, S], BF)
            sink_sb = perm.tile([D, S], BF)
            emat_sb = perm.tile([NB, NT * 128], F32)
            cmask_sb = perm.tile([128, 4 * 512], BF)
            ow_sb = perm.tile([D, G, HIDDEN], BF)

            nc.sync.dma_start(rot_sb[:], rot_d[:])
            nc.sync.dma_start(gwq_sb[:], gwq_d[:])
            nc.sync.dma_start(gwk_sb[:, 0, :], gwk_d[0:D, :])
            nc.sync.dma_start(gwk_sb[:, 1, :], gwk_d[D : 2 * D, :])
            nc.sync.dma_start(eye_sb[:], eye_d[:])
            nc.sync.dma_start(bcm_sb[:], bcm_d[:])
            nc.sync.dma_start(cosq_sb[:], cosq_d[:])
            nc.sync.dma_start(sinq_sb[:], sinq_d[:])
            nc.sync.dma_start(cosk_sb[:], cosk_d[:])
            nc.sync.dma_start(sink_sb[:], sink_d[:])
            nc.sync.dma_start(emat_sb[:], emat_d[:])
            nc.sync.dma_start(cmask_sb[:], cmask_d[:])
            for hh in range(G):
                nc.sync.dma_start(ow_sb[:, hh, :], ow_d[hh * D : (hh + 1) * D, :])
            nc.vector.memset(ones_sb[:], 1.0)

            # ---- phase 2: block gate (fp32) ----
            with tc.tile_pool(name="gp", bufs=1) as gp, tc.tile_pool(
                name="gps", bufs=1, space="PSUM"
            ) as gps, tc.tile_pool(name="gpsm", bufs=2, space="PSUM") as gpsm:
                t0 = gp.tile([D, NB], F32)
                qps = gp.tile([D, NB], F32)
                nc.vector.tensor_add(t0[:], qp_sb[:, 0, :], qp_sb[:, 1, :])
                nc.vector.tensor_add(qps[:], qp_sb[:, 2, :], qp_sb[:, 3, :])
                nc.vector.tensor_add(qps[:], t0[:], qps[:])

                kg_ps = gps.tile([NB, GH], F32)
                nc.tensor.matmul(kg_ps, km_sb[:], gwk_sb[:, 0, :], start=True, stop=False)
                nc.tensor.matmul(kg_ps, kx_sb[:], gwk_sb[:, 1, :], start=False, stop=True)
                qg_ps = gps.tile([NB, GH], F32)
                nc.tensor.matmul(qg_ps, qps[:], gwq_sb[:], start=True, stop=True)
                qg_sb = gp.tile([NB, GH], F32)
                kg_sb = gp.tile([NB, GH], F32)
                # fold mean-over-heads (1/G), block mean (1/BLK), GH^-0.5
                nc.scalar.mul(qg_sb[:], qg_ps[:], (1.0 / (G * BLK)) * GH**-0.5)
                nc.scalar.copy(kg_sb[:], kg_ps[:])

                qgT_ps = gps.tile([GH, NB], F32)
                nc.tensor.matmul(qgT_ps, qg_sb[:], eye_sb[:], start=True, stop=True)
                kgT_ps = gps.tile([GH, NB], F32)
                nc.tensor.matmul(kgT_ps, kg_sb[:], eye_sb[:], start=True, stop=True)
                qgT_sb = gp.tile([GH, NB], F32)
                kgT_sb = gp.tile([GH, NB], F32)
                nc.scalar.copy(qgT_sb[:], qgT_ps[:])
                nc.scalar.copy(kgT_sb[:], kgT_ps[:])

                lg_ps = gps.tile([NB, NB], F32)
                nc.tensor.matmul(lg_ps, qgT_sb[:], kgT_sb[:], start=True, stop=True)
                lg_sb = gp.tile([NB, NB], F32)
                nc.scalar.copy(lg_sb[:], lg_ps[:])
                lm_sb = gp.tile([NB, NB], F32)
                nc.vector.tensor_add(lm_sb[:], lg_sb[:], bcm_sb[:])
                ge_sb = gp.tile([NB, NB], F32)
                gsum = gp.tile([NB, 1], F32)
                nc.scalar.activation(ge_sb[:], lm_sb[:], AF.Exp, accum_out=gsum[:])
                grc = gp.tile([NB, 1], F32)
                nc.vector.reciprocal(grc[:], gsum[:])
                prob_sb = gp.tile([NB, NB], F32)
                nc.scalar.activation(prob_sb[:], ge_sb[:], AF.Copy, scale=grc[:])
                m01 = gp.tile([NB, NB], F32)
                nc.vector.tensor_scalar(m01[:], prob_sb[:], THR, None, op0=OP.is_ge)
                nc.vector.tensor_tensor(m01[:], m01[:], eye_sb[:], op=OP.max)
                # transpose: expansion partitions index k blocks, m01 rows
                # index q blocks
                m01t_ps = gps.tile([NB, NB], F32)
                nc.tensor.matmul(m01t_ps, m01[:], eye_sb[:], start=True, stop=True)
                m01t = gp.tile([NB, NB], F32)
                nc.scalar.copy(m01t[:], m01t_ps[:])

                if debug:
                    for nm, t in [
                        ("dlg", lg_sb),
                        ("dqg", qg_sb),
                        ("dkg", kg_sb),
                        ("dprob", prob_sb),
                        ("dm01", m01),
                    ]:
                        dd = nc.dram_tensor(
                            nm, list(t[:].shape), t[:].dtype, kind="ExternalOutput"
                        )
                        nc.sync.dma_start(dd[:], t[:])

                for i in range(NT):
                    mp = gpsm.tile([128, NB], F32)
                    nc.tensor.matmul(
                        mp,
                        emat_sb[:, i * 128 : (i + 1) * 128],
                        m01t[:],
                        start=True,
                        stop=True,
                    )
                    nc.scalar.copy(mask_sb[:, i, :], mp[:])

            # ---- phase 3: RoPE in place on q^T / k^T ----
            with tc.tile_pool(name="rp", bufs=4) as rp, tc.tile_pool(
                name="rps", bufs=4, space="PSUM"
            ) as rps:
                for hh in range(G + 1):
                    src = q_sb[:, hh, :] if hh < G else k_sb[:]
                    cs = cosq_sb if hh < G else cosk_sb
                    sn = sinq_sb if hh < G else sink_sb
                    for j in range(NS):
                        sl = slice(j * 512, (j + 1) * 512)
                        rt = rps.tile([D, 512], F32)
                        nc.tensor.matmul(rt, rot_sb[:], src[:, sl], start=True, stop=True)
                        t1 = rp.tile([D, 512], BF)
                        nc.vector.tensor_mul(t1[:], src[:, sl], cs[:, sl])
                        t2 = rp.tile([D, 512], BF)
                        nc.vector.tensor_mul(t2[:], rt[:], sn[:, sl])
                        nc.vector.tensor_add(src[:, sl], t1[:], t2[:])

            # ---- phase 4: masked attention (transposed P layout) ----
            from concourse.bass import AP

            with tc.tile_pool(name="ap_", bufs=4) as ap_, tc.tile_pool(
                name="sm", bufs=4
            ) as sm, tc.tile_pool(name="sps", bufs=3, space="PSUM") as sps, tc.tile_pool(
                name="pvs", bufs=2, space="PSUM"
            ) as pvs, tc.tile_pool(name="rbs", bufs=2, space="PSUM") as rbs:
                for hh in range(G):
                    for j in range(NS):
                        ssl = slice(j * 512, (j + 1) * 512)
                        pv_ps = pvs.tile([D + 1, 512], F32)
                        ntile = 4 * (j + 1)
                        for ti in range(ntile):
                            s_ps = sps.tile([128, 512], F32)
                            nc.tensor.matmul(
                                s_ps,
                                k_sb[:, ti * 128 : (ti + 1) * 128],
                                q_sb[:, hh, ssl],
                                start=True,
                                stop=True,
                                skip_group_check=True,
                            )
                            p_sb = ap_.tile([128, 512], BF)
                            nc.scalar.activation(p_sb[:], s_ps[:], AF.Exp)
                            if ti >= 4 * j:
                                r = ti - 4 * j
                                nc.vector.tensor_mul(
                                    p_sb[:],
                                    p_sb[:],
                                    cmask_sb[:, r * 512 : (r + 1) * 512],
                                )
                            msl = mask_sb[:, ti, j * 8 : (j + 1) * 8]
                            mb = AP(
                                tensor=msl.tensor,
                                offset=msl.offset,
                                ap=list(msl.ap) + [[0, BLK]],
                            )
                            p3 = p_sb[:].rearrange("p (b w) -> p b w", w=BLK)
                            nc.vector.tensor_tensor(p3, p3, mb, op=OP.mult)
                            nc.tensor.matmul(
                                pv_ps,
                                v_sb[:, ti, :],
                                p_sb[:],
                                start=(ti == 0),
                                stop=(ti == ntile - 1),
                                skip_group_check=True,
                            )
                        sr = sm.tile([1, 512], F32)
                        nc.scalar.copy(sr[:], pv_ps[D : D + 1, :])
                        rc = sm.tile([1, 512], F32)
                        nc.vector.reciprocal(rc[:], sr[:])
                        rcb = sm.tile([1, 512], BF)
                        nc.vector.tensor_copy(rcb[:], rc[:])
                        rb_ps = rbs.tile([D, 512], F32)
                        nc.tensor.matmul(
                            rb_ps, ones_sb[:, :D], rcb[:], start=True, stop=True
                        )
                        # HW: DVE may read only ONE input from PSUM
                        rb_sb = sm.tile([D, 512], F32)
                        nc.scalar.copy(rb_sb[:], rb_ps[:])
                        nc.vector.tensor_mul(
                            attn_sb[:, hh, ssl], pv_ps[:D, :], rb_sb[:]
                        )

            # ---- phase 5: o-projection partial ----
            with tc.tile_pool(name="op_", bufs=4) as op_, tc.tile_pool(
                name="ops", bufs=4, space="PSUM"
            ) as ops:
                for si in range(NT):
                    tsl = slice(si * 128, (si + 1) * 128)
                    for ej in range(NE):
                        esl = slice(ej * 512, (ej + 1) * 512)
                        o_ps = ops.tile([128, 512], F32)
                        for hh in range(G):
                            nc.tensor.matmul(
                                o_ps,
                                attn_sb[:, hh, tsl],
                                ow_sb[:, hh, esl],
                                start=(hh == 0),
                                stop=(hh == G - 1),
                            )
                        o_sb = op_.tile([128, 512], BF)
                        nc.scalar.copy(o_sb[:], o_ps[:])
                        nc.sync.dma_start(out_d[tsl, esl], o_sb[:])

            if debug:
                for nm, t in [
                    ("dq", q_sb),
                    ("dk", k_sb),
                    ("dv", v_sb),
                    ("dmask", mask_sb),
                    ("dqp", qp_sb),
                    ("dkm", km_sb),
                    ("dkx", kx_sb),
                    ("dattn", attn_sb),
                ]:
                    dd = nc.dram_tensor(
                        nm, list(t[:].shape), t[:].dtype, kind="ExternalOutput"
                    )
                    nc.sync.dma_start(dd[:], t[:])
    return nc


def _host_prep(hidden_states, cos, sin, qkv_w, o_w, gate_wq, gate_wk):
    bf = ml_dtypes.bfloat16
    X = np.asarray(hidden_states, np.float32).reshape(S, HIDDEN)
    qkv_w = np.asarray(qkv_w, np.float32)
    o_w = np.asarray(o_w, np.float32)
    cos = np.asarray(cos, np.float32)
    sin = np.asarray(sin, np.float32)

    xt = np.ascontiguousarray(X.T).astype(bf)
    scale = D**-0.5
    cosT = np.ascontiguousarray(cos.T)
    sinT = np.ascontiguousarray(sin.T)
    cosq = (cosT * scale).astype(bf)
    sinq = (sinT * scale).astype(bf)
    cosk = cosT.astype(bf)
    sink = sinT.astype(bf)

    rt = np.zeros((D, D), np.float32)
    h = D // 2
    rt[np.arange(h) + h, np.arange(h)] = -1.0
    rt[np.arange(h), np.arange(h) + h] = 1.0
    rt = rt.astype(bf)

    emat = np.zeros((NB, NT * 128), np.float32)
    for i in range(NT):
        for p in range(128):
            emat[2 * i + p // BLK, i * 128 + p] = 1.0
    eye = np.eye(NB, dtype=np.float32)

    bcm = np.where(
        np.arange(NB)[None, :] <= np.arange(NB)[:, None], 0.0, -60.0
    ).astype(np.float32)
    # cmask[p, r*512+col] = 1 if col - p >= 128*r (k token ti*128+p causal
    # w.r.t. q token j*512+col on diagonal tiles, r = ti - 4j)
    p_i = np.arange(128)[:, None]
    cmask = np.zeros((128, 4 * 512), np.float32)
    for r in range(4):
        col = np.arange(512)[None, :]
        cmask[:, r * 512 : (r + 1) * 512] = (col - p_i >= 128 * r).astype(
            np.float32
        )
    cmask = cmask.astype(bf)

    # k block mean is computed on-device as a SUM; fold 1/BLK into the
    # mean-pool half of gate_wk
    gwk_s = np.asarray(gate_wk, np.float32).copy()
    gwk_s[:D, :] *= 1.0 / BLK

    common = dict(
        xt=xt,
        cosq=cosq,
        sinq=sinq,
        cosk=cosk,
        sink=sink,
        rot=rt,
        gwq=np.asarray(gate_wq, np.float32),
        gwk=gwk_s,
        eye32=eye,
        emat=emat,
        bcm=bcm,
        cmask=cmask,
    )
    maps = []
    for c in range(NCORES):
        maps.append(
            dict(
                common,
                wq=qkv_w[:, c * G * D : (c + 1) * G * D].astype(bf),
                wk=qkv_w[:, H * D + c * D : H * D + (c + 1) * D].astype(bf),
                wv=qkv_w[
                    :, H * D + HK * D + c * D : H * D + HK * D + (c + 1) * D
                ].astype(bf),
                ow=o_w[c * G * D : (c + 1) * G * D, :].astype(bf),
            )
        )
    return maps


def _gather(results):
    acc = np.zeros((S, HIDDEN), np.float32)
    for r in results:
        acc += np.asarray(r["out_p"]).astype(np.float32)
    return acc.reshape(1, S, HIDDEN)


def _run(inputs, trace=False):
    global _prog
    if _prog is None:
        _prog = _build()
        if not _prog.is_finalized():
            _prog.finalize()
    from concourse import bass_utils

    maps = _host_prep(**inputs)
    res = bass_utils.run_bass_kernel_spmd(
        _prog, maps, list(range(NCORES)), trace=trace
    )
    return _gather(res.results), res


def kernel(**inputs):
    out, _ = _run(inputs, trace=False)
    return out
